# revision 1
# baseline (speedup 1.0000x reference)
"""Chamfer loss (brute-force, no sigma) on 8 trn2 NeuronCores.

Strategy (data-parallel over batch, one batch element per core):
  sq[m,n] = |src_m - dst_n|^2 is produced by ONE augmented matmul per tile:
     sq = L^T @ R,  K = 18 rows:
       rows 0-11 : exact 2-term bf16 split of -2*src_c x dst_c  (hi/lo cross)
       rows 12-14: ones (x) 3-term bf16 split of |dst_n|^2
       rows 15-17: 3-term bf16 split of |src_m|^2 (x) ones
  PE accumulates in fp32, so the full squared distance (small, >=0) is formed
  before any rounding.  ScalarE evacuates PSUM to SBUF as bf16; VectorE then
  does a 2x-mode TT-min fold chain + fused tensor_scalar min-reduce for the
  src->dst row minima, and a running elementwise TT-min into a [128, N]
  accumulator for the dst->src direction.  The dst->src partition-axis min is
  finished with a 32x32 stream transpose + reduce + two cross-quadrant folds
  (via tiny SBUF DMAs; DVE ops require equal base partitions).
  Host side only shards/preps inputs and takes sqrt/mean of the per-core
  minima.
"""

import numpy as np
import ml_dtypes
from contextlib import ExitStack

B, C = 8, 3
M = N = 4096
NCORES = 8
PB = 128          # output partition block (m rows per matmul)
KAUG = 18         # augmented contraction dim
BIG = 1.0e4       # > max possible squared distance (~150)
MMN = 512         # matmul moving free dim (one fp32 PSUM bank)
PW = 2048         # psum chunk width (fp32, 4 banks)
SB_BUFS = 4
VERSION = "fullblock-v4"

bf16np = ml_dtypes.bfloat16


# ----------------------------------------------------------------------------
# Device program
# ----------------------------------------------------------------------------

def _body(ctx, tc, lhs, rhs, rowmin_d, colmin_d, m, n, reps=1):
    import concourse.mybir as mybir

    nc = tc.nc
    f32 = mybir.dt.float32
    bf16 = mybir.dt.bfloat16
    MIN = mybir.AluOpType.min
    AX = mybir.AxisListType.X

    nblk = m // PB
    pw = min(PW, n)
    nch = n // pw
    mmn = min(MMN, pw)
    nq = pw // mmn
    nb32 = n // 32

    cpool = ctx.enter_context(tc.tile_pool(name="const", bufs=1))
    ppool = ctx.enter_context(tc.tile_pool(name="psum", bufs=2, space="PSUM"))
    spool = ctx.enter_context(tc.tile_pool(name="sb", bufs=SB_BUFS))
    rpool = ctx.enter_context(tc.tile_pool(name="scr", bufs=4))

    lhs_t = cpool.tile([KAUG, m], bf16)
    nc.sync.dma_start(out=lhs_t[:], in_=lhs[:])
    rhs_t = cpool.tile([KAUG, n], bf16)
    nc.sync.dma_start(out=rhs_t[:], in_=rhs[:])

    rowmin_t = cpool.tile([PB, nblk], f32)
    bacc = cpool.tile([PB, n], bf16)
    nc.vector.memset(bacc[:], BIG)

    for rep in range(reps):
        for i in range(nblk):
            sb = spool.tile([PB, n], bf16, tag="sb")
            for ch in range(nch):
                pt = ppool.tile([PB, pw], f32, tag="pt")
                for q in range(nq):
                    n0 = ch * pw + q * mmn
                    nc.tensor.matmul(
                        pt[:, q * mmn:(q + 1) * mmn],
                        lhs_t[:, i * PB:(i + 1) * PB],
                        rhs_t[:, n0:n0 + mmn],
                        start=True, stop=True,
                    )
                nc.scalar.copy(sb[:, ch * pw:(ch + 1) * pw], pt[:])

            # forward row-min: 2x-mode TT-min fold chain down to 512 wide,
            # then one fused tensor_scalar min-reduce into rowmin_t[:, i]
            src_t, w = sb, n
            lvl = 0
            while w > 512:
                nxt = rpool.tile([PB, w // 2], bf16, tag=f"fold{lvl}")
                nc.vector.tensor_tensor(nxt[:], src_t[:, :w // 2],
                                        src_t[:, w // 2:], MIN)
                src_t, w, lvl = nxt, w // 2, lvl + 1
            scr = rpool.tile([PB, w], bf16, tag="scr")
            nc.vector.tensor_scalar(scr[:], src_t[:], float(BIG), None, MIN,
                                    MIN, accum_out=rowmin_t[:, i:i + 1])

            # backward: running elementwise min over m-blocks
            nc.vector.tensor_tensor(bacc[:], bacc[:], sb[:], MIN)

    nc.sync.dma_start(out=rowmin_d[:], in_=rowmin_t[:])

    # dst->src: partition-axis min of bacc[p, nn] over p via 32x32 stream
    # transpose + in-block reduce + cross-quadrant folds (small SBUF DMAs).
    tr = rpool.tile([PB, n], bf16, tag="tr")
    nc.vector.transpose(tr[:], bacc[:])
    red = rpool.tile([PB, nb32], f32, tag="red")
    nc.vector.tensor_reduce(
        red[:], tr[:].rearrange("p (b i) -> p b i", i=32), AX, MIN)
    hi = rpool.tile([64, nb32], f32, tag="hi")
    nc.sync.dma_start(out=hi[:], in_=red[64:128, :])
    t1 = rpool.tile([64, nb32], f32, tag="t1")
    nc.vector.tensor_tensor(t1[:], red[0:64, :], hi[:], MIN)
    hi2 = rpool.tile([32, nb32], f32, tag="hi2")
    nc.sync.dma_start(out=hi2[:], in_=t1[32:64, :])
    colmin_sb = cpool.tile([32, nb32], f32)
    nc.vector.tensor_tensor(colmin_sb[:], t1[0:32, :], hi2[:], MIN)
    nc.sync.dma_start(out=colmin_d[:], in_=colmin_sb[:])


def build_nc(m=M, n=N, reps=1):
    import concourse.tile as tile
    import concourse.bacc as bacc_mod
    import concourse.mybir as mybir

    f32 = mybir.dt.float32
    bf16 = mybir.dt.bfloat16
    nblk = m // PB

    nc = bacc_mod.Bacc("TRN2", target_bir_lowering=False, debug=False)
    lhs = nc.dram_tensor("lhs_aug", [KAUG, m], bf16, kind="ExternalInput").ap()
    rhs = nc.dram_tensor("rhs_aug", [KAUG, n], bf16, kind="ExternalInput").ap()
    rowmin_d = nc.dram_tensor("rowmin", [PB, nblk], f32,
                              kind="ExternalOutput").ap()
    colmin_d = nc.dram_tensor("colmin", [32, n // 32], f32,
                              kind="ExternalOutput").ap()
    with tile.TileContext(nc) as tc:
        with ExitStack() as ctx:
            _body(ctx, tc, lhs, rhs, rowmin_d, colmin_d, m, n, reps=reps)
    nc.compile()
    return nc


# ----------------------------------------------------------------------------
# Host-side input prep: exact bf16 splits for the augmented operands
# ----------------------------------------------------------------------------

def _split2(x):
    """x (f64) -> (hi, lo) bf16 values returned as exact f64."""
    hi = x.astype(bf16np).astype(np.float64)
    lo = (x - hi).astype(bf16np).astype(np.float64)
    return hi, lo


def _split3(x):
    h = x.astype(bf16np).astype(np.float64)
    r = x - h
    mdl = r.astype(bf16np).astype(np.float64)
    l = (r - mdl).astype(bf16np).astype(np.float64)
    return h, mdl, l


def prep_inputs(pc_src, pc_dst):
    """Build per-batch augmented operands L, R: [B, 18, M/N] bf16."""
    s = np.asarray(pc_src, dtype=np.float64)   # [B, 3, M]
    d = np.asarray(pc_dst, dtype=np.float64)   # [B, 3, N]
    b = s.shape[0]
    m = s.shape[2]
    n = d.shape[2]

    s_hi, s_lo = _split2(s)
    d_hi, d_lo = _split2(d)
    s2 = ((s_hi + s_lo) ** 2).sum(axis=1)      # [B, M]
    d2 = ((d_hi + d_lo) ** 2).sum(axis=1)      # [B, N]
    s2h, s2m, s2l = _split3(s2)
    d2h, d2m, d2l = _split3(d2)

    L = np.zeros((b, KAUG, m), dtype=np.float64)
    R = np.zeros((b, KAUG, n), dtype=np.float64)
    L[:, 0:3] = -2.0 * s_hi
    R[:, 0:3] = d_hi
    L[:, 3:6] = -2.0 * s_hi
    R[:, 3:6] = d_lo
    L[:, 6:9] = -2.0 * s_lo
    R[:, 6:9] = d_hi
    L[:, 9:12] = -2.0 * s_lo
    R[:, 9:12] = d_lo
    L[:, 12:15] = 1.0
    R[:, 12] = d2h
    R[:, 13] = d2m
    R[:, 14] = d2l
    L[:, 15] = s2h
    L[:, 16] = s2m
    L[:, 17] = s2l
    R[:, 15:18] = 1.0
    return L.astype(bf16np), R.astype(bf16np)


# ----------------------------------------------------------------------------
# Cached PJRT runner (compile once, execute many)
# ----------------------------------------------------------------------------

_STATE = {}


def _get_runner(reps=1):
    key = (reps, VERSION, SB_BUFS)
    if key in _STATE:
        return _STATE[key]

    import jax
    from jax.experimental.shard_map import shard_map
    from jax.sharding import Mesh, PartitionSpec
    from concourse import bass2jax, mybir

    nc = build_nc(M, N, reps=reps)
    bass2jax.install_neuronx_cc_hook()

    in_names, out_names, out_avals = [], [], []
    for alloc in nc.m.functions[0].allocations:
        if not isinstance(alloc, mybir.MemoryLocationSet):
            continue
        name = alloc.memorylocations[0].name
        if alloc.kind == "ExternalInput":
            in_names.append(name)
        elif alloc.kind == "ExternalOutput":
            out_names.append(name)
            out_avals.append(jax.core.ShapedArray(
                tuple(alloc.tensor_shape), mybir.dt.np(alloc.dtype)))
    n_params = len(in_names)
    n_outs = len(out_names)
    all_in_names = tuple(in_names + out_names)
    donate = tuple(range(n_params, n_params + n_outs))

    def _jbody(*args):
        outs = bass2jax._bass_exec_p.bind(
            *args,
            out_avals=tuple(out_avals),
            in_names=all_in_names,
            out_names=tuple(out_names),
            lowering_input_output_aliases=(),
            sim_require_finite=True,
            sim_require_nnan=True,
            nc=nc,
        )
        return tuple(outs)

    devices = jax.devices()[:NCORES]
    mesh = Mesh(np.asarray(devices), ("core",))
    in_specs = (PartitionSpec("core"),) * (n_params + n_outs)
    out_specs = (PartitionSpec("core"),) * n_outs
    fn = jax.jit(
        shard_map(_jbody, mesh=mesh, in_specs=in_specs, out_specs=out_specs,
                  check_rep=False),
        donate_argnums=donate, keep_unused=True,
    )
    st = dict(fn=fn, nc=nc, in_names=in_names, out_names=out_names,
              out_avals=out_avals, n_params=n_params)
    _STATE[key] = st
    return st


def run_device(L, R, reps=1, _retry=True):
    """L, R: [NCORES, 18, M] bf16. Returns (rowmin[NCORES,128,M/128],
    colmin[NCORES,32,N/32]) squared-distance minima (fp32)."""
    st = _get_runner(reps)
    concat_in = []
    for name in st["in_names"]:
        arr = L if name == "lhs_aug" else R
        concat_in.append(np.concatenate([arr[c] for c in range(NCORES)], axis=0))
    concat_zeros = [
        np.zeros((NCORES * av.shape[0], *av.shape[1:]), av.dtype)
        for av in st["out_avals"]
    ]
    try:
        out_arrs = st["fn"](*concat_in, *concat_zeros)
        out_np = [np.asarray(a) for a in out_arrs]
    except Exception:
        # The shared axon terminal occasionally reports a transient
        # device-unrecoverable state; it clears after a short pause.
        if not _retry:
            raise
        import time as _time
        _time.sleep(20.0)
        return run_device(L, R, reps=reps, _retry=False)
    outs = {}
    for i, name in enumerate(st["out_names"]):
        av = st["out_avals"][i]
        outs[name] = out_np[i].reshape(NCORES, *av.shape)
    return outs["rowmin"], outs["colmin"]


# ----------------------------------------------------------------------------
# Public entry point
# ----------------------------------------------------------------------------

def _host_reduce(rowmin, colmin):
    # rowmin: [B, 128, M/128]; colmin: [B, 32, N/32]  (squared distances)
    fwd = np.sqrt(np.maximum(rowmin.astype(np.float64), 0.0)).mean()
    bwd = np.sqrt(np.maximum(colmin.astype(np.float64), 0.0)).mean()
    total = np.float32(fwd + bwd)
    return total


def kernel(pc_src, pc_dst):
    L, R = prep_inputs(pc_src, pc_dst)
    rowmin, colmin = run_device(L, R)
    total = _host_reduce(rowmin, colmin)
    return (total, total, total)



# revision 2
# speedup vs baseline: 1.9320x; 1.9320x over previous
"""Chamfer loss (brute-force, no sigma) on 8 trn2 NeuronCores.

Strategy (data-parallel over batch, one batch element per core):
  sq[m,n] = |src_m - dst_n|^2 is produced by ONE augmented matmul per tile:
     sq = L^T @ R,  K = 18 rows:
       rows 0-11 : exact 2-term bf16 split of -2*src_c x dst_c  (hi/lo cross)
       rows 12-14: ones (x) 3-term bf16 split of |dst_n|^2
       rows 15-17: 3-term bf16 split of |src_m|^2 (x) ones
  PE accumulates in fp32, so the full squared distance (small, >=0) is formed
  before any rounding.  Each [128, 2048] PSUM chunk is then consumed by two
  parallel engine streams in a per-chunk pipeline:
    - Act: PSUM -> SBUF bf16 evacuation (the only engine that can stream
      PSUM out at 1 elem/cycle/lane; ~122 us/core, the critical resource).
    - DVE: row-min via four fused tensor_scalar min+accum ops per chunk
      (4x mode; 512-wide accum_out - wider accums stall real HW), plus a
      running elementwise TT-min into the [128, 4096] col accumulator.
  Block 0 is evacuated directly into the accumulator (no memset, no TT).
  Tail: per column-half 32x32 stream transpose + TT-min halving of the
  32-groups, then one 128x128 DMA-xbar transpose + two TT-min halvings
  fold the partition quadrants (no partition-realign DMA hops).
  Host side only shards/preps inputs and takes sqrt/mean of the minima.
"""

import numpy as np
import ml_dtypes
from contextlib import ExitStack

B, C = 8, 3
M = N = 4096
NCORES = 8
PB = 128          # output partition block (m rows per matmul)
KAUG = 18         # augmented contraction dim
BIG = 1.0e4       # > max possible squared distance (~150)
MMN = 512         # matmul moving free dim (one fp32 PSUM bank)
PW = 2048         # psum chunk width (fp32, 4 banks)
SB_BUFS = 8
NBLK = M // PB

# Tunables.  SPLIT_EVAC: blocks whose chunk-1 evacuation runs as a fused DVE
# tensor_scalar straight from PSUM (1x; evac + row-min in one op) while Act
# handles chunk 0 — relieves the Act bottleneck at some DVE cost.
SPLIT_EVAC = ()
ROW_SPLIT = 4               # row-min accums per 2048-chunk ("ts" mode);
                            # 512-wide accum_out is the widest that is free
                            # on real HW (wider accums pay ~1us/op)
ROW_MODE = "ts"             # "ts": tensor_scalar+accum per chunk; "fold":
FOLD_TO = 512               #   TT-min halving chain down to FOLD_TO + accum
REDUCE_MODE = "ttfold"      # tail 32-group reduce: "reduce" | "ttfold"
SLOTS = 2 * (ROW_SPLIT if ROW_MODE == "ts" else 1)  # rowmin slots per block
VERSION = "balance-v3"

bf16np = ml_dtypes.bfloat16


# ----------------------------------------------------------------------------
# Device program
# ----------------------------------------------------------------------------

def _body(ctx, tc, lhs, rhs, rowmin_d, colmin_d, m, n, reps=1):
    import concourse.mybir as mybir

    nc = tc.nc
    f32 = mybir.dt.float32
    bf16 = mybir.dt.bfloat16
    MIN = mybir.AluOpType.min
    AX = mybir.AxisListType.X

    nblk = m // PB
    pw = min(PW, n)
    nch = n // pw
    mmn = min(MMN, pw)
    nq = pw // mmn

    split_evac = set(b for b in SPLIT_EVAC if b < nblk)
    assert 0 not in split_evac  # block 0 initializes bacc via Act

    cpool = ctx.enter_context(tc.tile_pool(name="const", bufs=1))
    ppool = ctx.enter_context(tc.tile_pool(name="psum", bufs=2, space="PSUM"))
    spool = ctx.enter_context(tc.tile_pool(name="sb", bufs=SB_BUFS))
    xpool = ctx.enter_context(tc.tile_pool(name="scr", bufs=2))
    rpool = ctx.enter_context(tc.tile_pool(name="red", bufs=2))

    lhs_t = cpool.tile([KAUG, m], bf16)
    nc.sync.dma_start(out=lhs_t[:], in_=lhs[:])
    rhs_t = cpool.tile([KAUG, n], bf16)
    nc.sync.dma_start(out=rhs_t[:], in_=rhs[:])

    rowmin_t = cpool.tile([PB, SLOTS * nblk], f32)
    nc.vector.memset(rowmin_t[:], BIG)
    bacc = cpool.tile([PB, n], bf16)

    def row_ts(dchunk, i, ch):
        """Row-min of an evacuated bf16 chunk: fused TS min-reduce at 4x
        (optionally after a TT fold chain, or split into sub-accums)."""
        base = SLOTS * i + ch * (SLOTS // nch)
        if ROW_MODE == "fold":
            src_t, w = dchunk, pw
            lvl = 0
            while w > FOLD_TO:
                nxt = xpool.tile([PB, w // 2], bf16, tag=f"fold{lvl}")
                nc.vector.tensor_tensor(nxt[:], src_t[:, :w // 2],
                                        src_t[:, w // 2:w], MIN)
                src_t, w, lvl = nxt, w // 2, lvl + 1
            scr = xpool.tile([PB, w], bf16, tag="scrf")
            nc.vector.tensor_scalar(
                scr[:], src_t[:, 0:w], float(BIG), None, MIN, MIN,
                accum_out=rowmin_t[:, base:base + 1])
        else:
            scr = xpool.tile([PB, pw], bf16, tag="scr")
            rw = pw // ROW_SPLIT
            for s in range(ROW_SPLIT):
                nc.vector.tensor_scalar(
                    scr[:, s * rw:(s + 1) * rw],
                    dchunk[:, s * rw:(s + 1) * rw], float(BIG), None, MIN,
                    MIN, accum_out=rowmin_t[:, base + s:base + s + 1])

    red_both = cpool.tile([PB, (n // pw) * (pw // 32)], bf16)

    def tail_half(ch):
        """32-group partition min of bacc[:, chunk ch] -> red_both slice."""
        w = pw
        nb = w // 32
        csl = slice(ch * pw, (ch + 1) * pw)
        tr = rpool.tile([PB, w], bf16, tag=f"tr{ch}")
        nc.vector.transpose(tr[:], bacc[:, csl])
        red = red_both[:, ch * nb:(ch + 1) * nb]
        if REDUCE_MODE == "ttfold":
            srcv = tr[:].rearrange("p (b i) -> p b i", i=32)
            wi, lvl = 32, 0
            while wi > 1:
                half = wi // 2
                if half > 1:
                    dst = rpool.tile([PB, nb * half], bf16,
                                     tag=f"tf{ch}_{lvl}",
                                     name=f"tf{ch}_{lvl}")[:]
                else:
                    dst = red
                dstv = dst.rearrange("p (b i) -> p b i", i=half)
                nc.vector.tensor_tensor(dstv, srcv[:, :, 0:half],
                                        srcv[:, :, half:wi], MIN)
                srcv, wi, lvl = dstv, half, lvl + 1
        else:
            nc.vector.tensor_reduce(
                red, tr[:].rearrange("p (b i) -> p b i", i=32), AX, MIN)

    def tail_fold():
        """Cross-quadrant fold of red_both: one 128x128 DMA-xbar transpose
        puts the partition quadrants on the free axis, then two TT-min
        halvings finish the per-column minima."""
        redT = rpool.tile([PB, PB], bf16, tag="redT")
        nc.sync.dma_start_transpose(out=redT[:], in_=red_both[:])
        t1 = rpool.tile([PB, 64], bf16, tag="t1")
        nc.vector.tensor_tensor(t1[:], redT[:, 0:64], redT[:, 64:128], MIN)
        cm = rpool.tile([PB, 32], f32, tag="cm")
        nc.vector.tensor_tensor(cm[:], t1[:, 0:32], t1[:, 32:64], MIN)
        nc.sync.dma_start(out=colmin_d[:], in_=cm[:])

    for rep in range(reps):
        for i in range(nblk):
            last = i == nblk - 1 and rep == reps - 1
            for ch in range(nch):
                pt = ppool.tile([PB, pw], f32, tag="pt")
                for q in range(nq):
                    n0 = ch * pw + q * mmn
                    nc.tensor.matmul(
                        pt[:, q * mmn:(q + 1) * mmn],
                        lhs_t[:, i * PB:(i + 1) * PB],
                        rhs_t[:, n0:n0 + mmn],
                        start=True, stop=True,
                    )
                if i == 0:
                    dchunk = bacc[:, ch * pw:(ch + 1) * pw]
                else:
                    dchunk = spool.tile([PB, pw], bf16, tag="sb")
                if i in split_evac and ch == 1:
                    # fused evac + row-min in one 1x DVE op (relieves Act)
                    base = SLOTS * i + SLOTS // nch
                    nc.vector.tensor_scalar(
                        dchunk[:], pt[:], float(BIG), None, MIN, MIN,
                        accum_out=rowmin_t[:, base:base + 1])
                else:
                    nc.scalar.copy(dchunk[:], pt[:])
                    row_ts(dchunk, i, ch)
                if i > 0:
                    csl = slice(ch * pw, (ch + 1) * pw)
                    nc.vector.tensor_tensor(bacc[:, csl], bacc[:, csl],
                                            dchunk[:], MIN)
                if last and ch == 0:
                    tail_half(0)
            if last:
                tail_half(1)
                tail_fold()

    nc.sync.dma_start(out=rowmin_d[:], in_=rowmin_t[:])


def build_nc(m=M, n=N, reps=1):
    import concourse.tile as tile
    import concourse.bacc as bacc_mod
    import concourse.mybir as mybir

    f32 = mybir.dt.float32
    bf16 = mybir.dt.bfloat16
    nblk = m // PB

    nc = bacc_mod.Bacc("TRN2", target_bir_lowering=False, debug=False)
    lhs = nc.dram_tensor("lhs_aug", [KAUG, m], bf16, kind="ExternalInput").ap()
    rhs = nc.dram_tensor("rhs_aug", [KAUG, n], bf16, kind="ExternalInput").ap()
    rowmin_d = nc.dram_tensor("rowmin", [PB, SLOTS * nblk], f32,
                              kind="ExternalOutput").ap()
    colmin_d = nc.dram_tensor("colmin", [PB, 32], f32,
                              kind="ExternalOutput").ap()
    with tile.TileContext(nc) as tc:
        with ExitStack() as ctx:
            _body(ctx, tc, lhs, rhs, rowmin_d, colmin_d, m, n, reps=reps)
    nc.compile()
    return nc


# ----------------------------------------------------------------------------
# Host-side input prep: exact bf16 splits for the augmented operands
# ----------------------------------------------------------------------------

def _split2(x):
    hi = x.astype(bf16np).astype(np.float64)
    lo = (x - hi).astype(bf16np).astype(np.float64)
    return hi, lo


def _split3(x):
    h = x.astype(bf16np).astype(np.float64)
    r = x - h
    mdl = r.astype(bf16np).astype(np.float64)
    l = (r - mdl).astype(bf16np).astype(np.float64)
    return h, mdl, l


def prep_inputs(pc_src, pc_dst):
    """Build per-batch augmented operands L, R: [B, 18, M/N] bf16."""
    s = np.asarray(pc_src, dtype=np.float64)   # [B, 3, M]
    d = np.asarray(pc_dst, dtype=np.float64)   # [B, 3, N]
    b = s.shape[0]
    m = s.shape[2]
    n = d.shape[2]

    s_hi, s_lo = _split2(s)
    d_hi, d_lo = _split2(d)
    s2 = ((s_hi + s_lo) ** 2).sum(axis=1)      # [B, M]
    d2 = ((d_hi + d_lo) ** 2).sum(axis=1)      # [B, N]
    s2h, s2m, s2l = _split3(s2)
    d2h, d2m, d2l = _split3(d2)

    L = np.zeros((b, KAUG, m), dtype=np.float64)
    R = np.zeros((b, KAUG, n), dtype=np.float64)
    L[:, 0:3] = -2.0 * s_hi
    R[:, 0:3] = d_hi
    L[:, 3:6] = -2.0 * s_hi
    R[:, 3:6] = d_lo
    L[:, 6:9] = -2.0 * s_lo
    R[:, 6:9] = d_hi
    L[:, 9:12] = -2.0 * s_lo
    R[:, 9:12] = d_lo
    L[:, 12:15] = 1.0
    R[:, 12] = d2h
    R[:, 13] = d2m
    R[:, 14] = d2l
    L[:, 15] = s2h
    L[:, 16] = s2m
    L[:, 17] = s2l
    R[:, 15:18] = 1.0
    return L.astype(bf16np), R.astype(bf16np)


# ----------------------------------------------------------------------------
# Cached PJRT runner (compile once, execute many)
# ----------------------------------------------------------------------------

_STATE = {}


def _get_runner(reps=1):
    key = (reps, VERSION, SB_BUFS, SPLIT_EVAC, ROW_MODE, ROW_SPLIT,
           FOLD_TO, REDUCE_MODE)
    if key in _STATE:
        return _STATE[key]

    import jax
    from jax.experimental.shard_map import shard_map
    from jax.sharding import Mesh, PartitionSpec
    from concourse import bass2jax, mybir

    nc = build_nc(M, N, reps=reps)
    bass2jax.install_neuronx_cc_hook()

    in_names, in_shapes, out_names, out_avals = [], {}, [], []
    for alloc in nc.m.functions[0].allocations:
        if not isinstance(alloc, mybir.MemoryLocationSet):
            continue
        name = alloc.memorylocations[0].name
        if alloc.kind == "ExternalInput":
            in_names.append(name)
            in_shapes[name] = (tuple(alloc.tensor_shape),
                               mybir.dt.np(alloc.dtype))
        elif alloc.kind == "ExternalOutput":
            out_names.append(name)
            out_avals.append(jax.core.ShapedArray(
                tuple(alloc.tensor_shape), mybir.dt.np(alloc.dtype)))
    n_params = len(in_names)
    n_outs = len(out_names)
    all_in_names = tuple(in_names + out_names)
    donate = tuple(range(n_params, n_params + n_outs))

    def _jbody(*args):
        outs = bass2jax._bass_exec_p.bind(
            *args,
            out_avals=tuple(out_avals),
            in_names=all_in_names,
            out_names=tuple(out_names),
            lowering_input_output_aliases=(),
            sim_require_finite=True,
            sim_require_nnan=True,
            nc=nc,
        )
        return tuple(outs)

    devices = jax.devices()[:NCORES]
    mesh = Mesh(np.asarray(devices), ("core",))
    in_specs = (PartitionSpec("core"),) * (n_params + n_outs)
    out_specs = (PartitionSpec("core"),) * n_outs
    fn = jax.jit(
        shard_map(_jbody, mesh=mesh, in_specs=in_specs, out_specs=out_specs,
                  check_rep=False),
        donate_argnums=donate, keep_unused=True,
    )
    st = dict(fn=fn, nc=nc, in_names=in_names, in_shapes=in_shapes,
              out_names=out_names, out_avals=out_avals, n_params=n_params)
    _STATE[key] = st
    return st


def run_device(L, R, reps=1, _retry=True):
    """L, R: [NCORES, 18, M] bf16. Returns (rowmin[NCORES,128,2*M/128],
    colmin[NCORES,32,N/32]) squared-distance minima (fp32)."""
    st = _get_runner(reps)
    concat_in = []
    for name in st["in_names"]:
        if name == "lhs_aug":
            arr = L
        elif name == "rhs_aug":
            arr = R
        else:  # e.g. partition_id — framework-added aux input
            shape, dt = st["in_shapes"][name]
            concat_in.append(np.zeros((NCORES * shape[0], *shape[1:]), dt))
            continue
        concat_in.append(np.concatenate([arr[c] for c in range(NCORES)], axis=0))
    concat_zeros = [
        np.zeros((NCORES * av.shape[0], *av.shape[1:]), av.dtype)
        for av in st["out_avals"]
    ]
    try:
        out_arrs = st["fn"](*concat_in, *concat_zeros)
        out_np = [np.asarray(a) for a in out_arrs]
    except Exception:
        # The shared axon terminal occasionally reports a transient
        # device-unrecoverable state; it clears after a short pause.
        if not _retry:
            raise
        import time as _time
        _time.sleep(20.0)
        return run_device(L, R, reps=reps, _retry=False)
    outs = {}
    for i, name in enumerate(st["out_names"]):
        av = st["out_avals"][i]
        outs[name] = out_np[i].reshape(NCORES, *av.shape)
    return outs["rowmin"], outs["colmin"]


# ----------------------------------------------------------------------------
# Public entry point
# ----------------------------------------------------------------------------

def _host_reduce(rowmin, colmin):
    # rowmin: [B, 128, 2*nblk] (per-half-chunk row minima); colmin: [B, 32,
    # N/32] (squared distances)
    b = rowmin.shape[0]
    rm = rowmin.reshape(b, PB, NBLK, -1).min(axis=3)
    fwd = np.sqrt(np.maximum(rm.astype(np.float64), 0.0)).mean()
    bwd = np.sqrt(np.maximum(colmin.astype(np.float64), 0.0)).mean()
    total = np.float32(fwd + bwd)
    return total


def kernel(pc_src, pc_dst):
    L, R = prep_inputs(pc_src, pc_dst)
    rowmin, colmin = run_device(L, R)
    total = _host_reduce(rowmin, colmin)
    return (total, total, total)


# revision 3
# speedup vs baseline: 1.9710x; 1.0202x over previous
"""Chamfer loss (brute-force, no sigma) on 8 trn2 NeuronCores.

Strategy (data-parallel over batch, one batch element per core):
  sq[m,n] = |src_m - dst_n|^2 is produced by ONE augmented matmul per tile:
     sq = L^T @ R,  K = 18 rows:
       rows 0-11 : exact 2-term bf16 split of -2*src_c x dst_c  (hi/lo cross)
       rows 12-14: ones (x) 3-term bf16 split of |dst_n|^2
       rows 15-17: 3-term bf16 split of |src_m|^2 (x) ones
  PE accumulates in fp32, so the full squared distance (small, >=0) is formed
  before any rounding.  Each [128, 2048] PSUM chunk then flows through two
  parallel engine streams in a per-chunk pipeline:
    - Act: PSUM -> SBUF bf16 evacuation (the only engine that can stream
      PSUM out at 1 elem/cycle/lane; ~122 us/core).
    - DVE: row-min via four fused tensor_scalar min+accum ops per chunk
      (4x mode; 512-wide accum_out - wider accums stall real HW), plus one
      full-width running TT-min per block into the [128, 4096] col
      accumulator (split per-chunk for the last block so the tail starts
      early).  Block 0 is evacuated directly into the accumulator.
  Inputs are DMA'd in split pieces (first block/chunk operands first) so the
  PE starts ~2.5us earlier.  Tail: per column-half 32x32 stream transpose +
  TT-min halving of the 32-groups into a [128, 128] tile that is shipped to
  the host, which does the final cross-quadrant fold + sqrt/mean (avoids a
  serial DMA-transpose + fold chain at the very end of the kernel).
"""

import numpy as np
import ml_dtypes
from contextlib import ExitStack

B, C = 8, 3
M = N = 4096
NCORES = 8
PB = 128          # output partition block (m rows per matmul)
KAUG = 18         # augmented contraction dim
BIG = 1.0e4       # > max possible squared distance (~150)
MMN = 512         # matmul moving free dim (one fp32 PSUM bank)
PW = 2048         # psum chunk width (fp32, 4 banks)
SB_BUFS = 8
NBLK = M // PB

# Tunables.  SPLIT_EVAC: blocks whose chunk-1 evacuation runs as a fused DVE
# tensor_scalar straight from PSUM (1x; evac + row-min in one op) while Act
# handles chunk 0 — relieves the Act bottleneck at some DVE cost.
SPLIT_EVAC = ()
ROW_SPLIT = 4               # row-min accums per 2048-chunk ("ts" mode);
                            # 512-wide accum_out is the widest that is free
                            # on real HW (wider accums pay ~1us/op)
ROW_MODE = "ts"             # "ts": tensor_scalar+accum per chunk; "fold":
FOLD_TO = 512               #   TT-min halving chain down to FOLD_TO + accum
REDUCE_MODE = "ttfold"      # tail 32-group reduce: "reduce" | "ttfold"
SLOTS = 2 * (ROW_SPLIT if ROW_MODE == "ts" else 1)  # rowmin slots per block
VERSION = "balance-v3"

bf16np = ml_dtypes.bfloat16


# ----------------------------------------------------------------------------
# Device program
# ----------------------------------------------------------------------------

def _body(ctx, tc, lhs, rhs, rowmin_d, colmin_d, m, n, reps=1):
    import concourse.mybir as mybir

    nc = tc.nc
    f32 = mybir.dt.float32
    bf16 = mybir.dt.bfloat16
    MIN = mybir.AluOpType.min
    AX = mybir.AxisListType.X

    nblk = m // PB
    pw = min(PW, n)
    nch = n // pw
    mmn = min(MMN, pw)
    nq = pw // mmn

    split_evac = set(b for b in SPLIT_EVAC if b < nblk)
    assert 0 not in split_evac  # block 0 initializes bacc via Act

    cpool = ctx.enter_context(tc.tile_pool(name="const", bufs=1))
    ppool = ctx.enter_context(tc.tile_pool(name="psum", bufs=2, space="PSUM"))
    spool = ctx.enter_context(tc.tile_pool(name="sb", bufs=SB_BUFS))
    xpool = ctx.enter_context(tc.tile_pool(name="scr", bufs=2))
    rpool = ctx.enter_context(tc.tile_pool(name="red", bufs=2))

    # Split the input DMAs so block-0/chunk-0 operands land first and the
    # PE can start ~2.5us earlier than with monolithic transfers.
    lhs_t = cpool.tile([KAUG, m], bf16)
    nc.gpsimd.dma_start(out=lhs_t[:, 0:PB], in_=lhs[:, 0:PB])
    rhs_t = cpool.tile([KAUG, n], bf16)
    nc.sync.dma_start(out=rhs_t[:, 0:mmn], in_=rhs[:, 0:mmn])
    nc.sync.dma_start(out=rhs_t[:, mmn:pw], in_=rhs[:, mmn:pw])
    nc.sync.dma_start(out=lhs_t[:, PB:m], in_=lhs[:, PB:m])
    nc.sync.dma_start(out=rhs_t[:, pw:n], in_=rhs[:, pw:n])

    rowmin_t = cpool.tile([PB, SLOTS * nblk], f32)
    nc.vector.memset(rowmin_t[:], BIG)
    bacc = cpool.tile([PB, n], bf16)

    def row_ts(dchunk, i, ch):
        """Row-min of an evacuated bf16 chunk: fused TS min-reduce at 4x
        (optionally after a TT fold chain, or split into sub-accums)."""
        base = SLOTS * i + ch * (SLOTS // nch)
        if ROW_MODE == "fold":
            src_t, w = dchunk, pw
            lvl = 0
            while w > FOLD_TO:
                nxt = xpool.tile([PB, w // 2], bf16, tag=f"fold{lvl}")
                nc.vector.tensor_tensor(nxt[:], src_t[:, :w // 2],
                                        src_t[:, w // 2:w], MIN)
                src_t, w, lvl = nxt, w // 2, lvl + 1
            scr = xpool.tile([PB, w], bf16, tag="scrf")
            nc.vector.tensor_scalar(
                scr[:], src_t[:, 0:w], float(BIG), None, MIN, MIN,
                accum_out=rowmin_t[:, base:base + 1])
        else:
            scr = xpool.tile([PB, pw], bf16, tag="scr")
            rw = pw // ROW_SPLIT
            for s in range(ROW_SPLIT):
                nc.vector.tensor_scalar(
                    scr[:, s * rw:(s + 1) * rw],
                    dchunk[:, s * rw:(s + 1) * rw], float(BIG), None, MIN,
                    MIN, accum_out=rowmin_t[:, base + s:base + s + 1])

    red_both = cpool.tile([PB, (n // pw) * (pw // 32)], bf16)

    def tail_half(ch):
        """32-group partition min of bacc[:, chunk ch] -> red_both slice."""
        w = pw
        nb = w // 32
        csl = slice(ch * pw, (ch + 1) * pw)
        tr = rpool.tile([PB, w], bf16, tag=f"tr{ch}")
        nc.vector.transpose(tr[:], bacc[:, csl])
        red = red_both[:, ch * nb:(ch + 1) * nb]
        if REDUCE_MODE == "ttfold":
            srcv = tr[:].rearrange("p (b i) -> p b i", i=32)
            wi, lvl = 32, 0
            while wi > 1:
                half = wi // 2
                if half > 1:
                    dst = rpool.tile([PB, nb * half], bf16,
                                     tag=f"tf{ch}_{lvl}",
                                     name=f"tf{ch}_{lvl}")[:]
                else:
                    dst = red
                dstv = dst.rearrange("p (b i) -> p b i", i=half)
                nc.vector.tensor_tensor(dstv, srcv[:, :, 0:half],
                                        srcv[:, :, half:wi], MIN)
                srcv, wi, lvl = dstv, half, lvl + 1
        else:
            nc.vector.tensor_reduce(
                red, tr[:].rearrange("p (b i) -> p b i", i=32), AX, MIN)


    for rep in range(reps):
        for i in range(nblk):
            last = i == nblk - 1 and rep == reps - 1
            for ch in range(nch):
                pt = ppool.tile([PB, pw], f32, tag="pt")
                for q in range(nq):
                    n0 = ch * pw + q * mmn
                    nc.tensor.matmul(
                        pt[:, q * mmn:(q + 1) * mmn],
                        lhs_t[:, i * PB:(i + 1) * PB],
                        rhs_t[:, n0:n0 + mmn],
                        start=True, stop=True,
                    )
                if i == 0:
                    dest = bacc
                elif ch == 0:
                    dest = spool.tile([PB, n], bf16, tag="sb")
                dchunk = dest[:, ch * pw:(ch + 1) * pw]
                if i in split_evac and ch == 1:
                    # fused evac + row-min in one 1x DVE op (relieves Act)
                    base = SLOTS * i + SLOTS // nch
                    nc.vector.tensor_scalar(
                        dchunk[:], pt[:], float(BIG), None, MIN, MIN,
                        accum_out=rowmin_t[:, base:base + 1])
                else:
                    nc.scalar.copy(dchunk[:], pt[:])
                    row_ts(dchunk, i, ch)
                if i > 0 and (last or ch == nch - 1):
                    if last:
                        # per-chunk col TT so tail_half(0) can start early
                        csl = slice(ch * pw, (ch + 1) * pw)
                        nc.vector.tensor_tensor(bacc[:, csl], bacc[:, csl],
                                                dchunk[:], MIN)
                    else:
                        # one merged full-width col TT (fewer DVE ops)
                        nc.vector.tensor_tensor(bacc[:], bacc[:], dest[:],
                                                MIN)
                if last and ch == 0:
                    tail_half(0)
            if last:
                nc.sync.dma_start(out=rowmin_d[:], in_=rowmin_t[:])
                tail_half(1)
                # the cross-quadrant fold of red_both happens on the host;
                # shipping the [128, 128] pre-fold tile avoids a serial
                # DMA-transpose + fold chain at the very end of the kernel.
                nc.sync.dma_start(out=colmin_d[:], in_=red_both[:])


def build_nc(m=M, n=N, reps=1):
    import concourse.tile as tile
    import concourse.bacc as bacc_mod
    import concourse.mybir as mybir

    f32 = mybir.dt.float32
    bf16 = mybir.dt.bfloat16
    nblk = m // PB

    nc = bacc_mod.Bacc("TRN2", target_bir_lowering=False, debug=False)
    lhs = nc.dram_tensor("lhs_aug", [KAUG, m], bf16, kind="ExternalInput").ap()
    rhs = nc.dram_tensor("rhs_aug", [KAUG, n], bf16, kind="ExternalInput").ap()
    rowmin_d = nc.dram_tensor("rowmin", [PB, SLOTS * nblk], f32,
                              kind="ExternalOutput").ap()
    colmin_d = nc.dram_tensor("colmin", [PB, PB], bf16,
                              kind="ExternalOutput").ap()
    with tile.TileContext(nc) as tc:
        with ExitStack() as ctx:
            _body(ctx, tc, lhs, rhs, rowmin_d, colmin_d, m, n, reps=reps)
    nc.compile()
    return nc


# ----------------------------------------------------------------------------
# Host-side input prep: exact bf16 splits for the augmented operands
# ----------------------------------------------------------------------------

def _split2(x):
    hi = x.astype(bf16np).astype(np.float64)
    lo = (x - hi).astype(bf16np).astype(np.float64)
    return hi, lo


def _split3(x):
    h = x.astype(bf16np).astype(np.float64)
    r = x - h
    mdl = r.astype(bf16np).astype(np.float64)
    l = (r - mdl).astype(bf16np).astype(np.float64)
    return h, mdl, l


def prep_inputs(pc_src, pc_dst):
    """Build per-batch augmented operands L, R: [B, 18, M/N] bf16."""
    s = np.asarray(pc_src, dtype=np.float64)   # [B, 3, M]
    d = np.asarray(pc_dst, dtype=np.float64)   # [B, 3, N]
    b = s.shape[0]
    m = s.shape[2]
    n = d.shape[2]

    s_hi, s_lo = _split2(s)
    d_hi, d_lo = _split2(d)
    s2 = ((s_hi + s_lo) ** 2).sum(axis=1)      # [B, M]
    d2 = ((d_hi + d_lo) ** 2).sum(axis=1)      # [B, N]
    s2h, s2m, s2l = _split3(s2)
    d2h, d2m, d2l = _split3(d2)

    L = np.zeros((b, KAUG, m), dtype=np.float64)
    R = np.zeros((b, KAUG, n), dtype=np.float64)
    L[:, 0:3] = -2.0 * s_hi
    R[:, 0:3] = d_hi
    L[:, 3:6] = -2.0 * s_hi
    R[:, 3:6] = d_lo
    L[:, 6:9] = -2.0 * s_lo
    R[:, 6:9] = d_hi
    L[:, 9:12] = -2.0 * s_lo
    R[:, 9:12] = d_lo
    L[:, 12:15] = 1.0
    R[:, 12] = d2h
    R[:, 13] = d2m
    R[:, 14] = d2l
    L[:, 15] = s2h
    L[:, 16] = s2m
    L[:, 17] = s2l
    R[:, 15:18] = 1.0
    return L.astype(bf16np), R.astype(bf16np)


# ----------------------------------------------------------------------------
# Cached PJRT runner (compile once, execute many)
# ----------------------------------------------------------------------------

_STATE = {}


def _get_runner(reps=1):
    key = (reps, VERSION, SB_BUFS, SPLIT_EVAC, ROW_MODE, ROW_SPLIT,
           FOLD_TO, REDUCE_MODE)
    if key in _STATE:
        return _STATE[key]

    import jax
    from jax.experimental.shard_map import shard_map
    from jax.sharding import Mesh, PartitionSpec
    from concourse import bass2jax, mybir

    nc = build_nc(M, N, reps=reps)
    bass2jax.install_neuronx_cc_hook()

    in_names, in_shapes, out_names, out_avals = [], {}, [], []
    for alloc in nc.m.functions[0].allocations:
        if not isinstance(alloc, mybir.MemoryLocationSet):
            continue
        name = alloc.memorylocations[0].name
        if alloc.kind == "ExternalInput":
            in_names.append(name)
            in_shapes[name] = (tuple(alloc.tensor_shape),
                               mybir.dt.np(alloc.dtype))
        elif alloc.kind == "ExternalOutput":
            out_names.append(name)
            out_avals.append(jax.core.ShapedArray(
                tuple(alloc.tensor_shape), mybir.dt.np(alloc.dtype)))
    n_params = len(in_names)
    n_outs = len(out_names)
    all_in_names = tuple(in_names + out_names)
    donate = tuple(range(n_params, n_params + n_outs))

    def _jbody(*args):
        outs = bass2jax._bass_exec_p.bind(
            *args,
            out_avals=tuple(out_avals),
            in_names=all_in_names,
            out_names=tuple(out_names),
            lowering_input_output_aliases=(),
            sim_require_finite=True,
            sim_require_nnan=True,
            nc=nc,
        )
        return tuple(outs)

    devices = jax.devices()[:NCORES]
    mesh = Mesh(np.asarray(devices), ("core",))
    in_specs = (PartitionSpec("core"),) * (n_params + n_outs)
    out_specs = (PartitionSpec("core"),) * n_outs
    fn = jax.jit(
        shard_map(_jbody, mesh=mesh, in_specs=in_specs, out_specs=out_specs,
                  check_rep=False),
        donate_argnums=donate, keep_unused=True,
    )
    st = dict(fn=fn, nc=nc, in_names=in_names, in_shapes=in_shapes,
              out_names=out_names, out_avals=out_avals, n_params=n_params)
    _STATE[key] = st
    return st


def run_device(L, R, reps=1, _retry=True):
    """L, R: [NCORES, 18, M] bf16. Returns (rowmin[NCORES,128,2*M/128],
    colmin[NCORES,32,N/32]) squared-distance minima (fp32)."""
    st = _get_runner(reps)
    concat_in = []
    for name in st["in_names"]:
        if name == "lhs_aug":
            arr = L
        elif name == "rhs_aug":
            arr = R
        else:  # e.g. partition_id — framework-added aux input
            shape, dt = st["in_shapes"][name]
            concat_in.append(np.zeros((NCORES * shape[0], *shape[1:]), dt))
            continue
        concat_in.append(np.concatenate([arr[c] for c in range(NCORES)], axis=0))
    concat_zeros = [
        np.zeros((NCORES * av.shape[0], *av.shape[1:]), av.dtype)
        for av in st["out_avals"]
    ]
    try:
        out_arrs = st["fn"](*concat_in, *concat_zeros)
        out_np = [np.asarray(a) for a in out_arrs]
    except Exception:
        # The shared axon terminal occasionally reports a transient
        # device-unrecoverable state; it clears after a short pause.
        if not _retry:
            raise
        import time as _time
        _time.sleep(20.0)
        return run_device(L, R, reps=reps, _retry=False)
    outs = {}
    for i, name in enumerate(st["out_names"]):
        av = st["out_avals"][i]
        outs[name] = out_np[i].reshape(NCORES, *av.shape)
    return outs["rowmin"], outs["colmin"]


# ----------------------------------------------------------------------------
# Public entry point
# ----------------------------------------------------------------------------

def _host_reduce(rowmin, colmin):
    # rowmin: [B, 128, 2*nblk] (per-half-chunk row minima); colmin: [B, 32,
    # N/32] (squared distances)
    b = rowmin.shape[0]
    rm = rowmin.reshape(b, PB, NBLK, -1).min(axis=3)
    fwd = np.sqrt(np.maximum(rm.astype(np.float64), 0.0)).mean()
    # colmin arrives pre-fold as [128, 128]: partition = (quadrant, s),
    # free = 32-column-group; fold the 4 partition quadrants here.
    cq = colmin.astype(np.float64).reshape(b, 4, 32, PB).min(axis=1)
    bwd = np.sqrt(np.maximum(cq, 0.0)).mean()
    total = np.float32(fwd + bwd)
    return total


def kernel(pc_src, pc_dst):
    L, R = prep_inputs(pc_src, pc_dst)
    rowmin, colmin = run_device(L, R)
    total = _host_reduce(rowmin, colmin)
    return (total, total, total)


# revision 4
# speedup vs baseline: 1.9789x; 1.0040x over previous
"""Chamfer loss (brute-force, no sigma) on 8 trn2 NeuronCores.

Strategy (data-parallel over batch, one batch element per core):
  sq[m,n] = |src_m - dst_n|^2 is produced by ONE augmented matmul per tile:
     sq = L^T @ R,  K = 18 rows:
       rows 0-11 : exact 2-term bf16 split of -2*src_c x dst_c  (hi/lo cross)
       rows 12-14: ones (x) 3-term bf16 split of |dst_n|^2
       rows 15-17: 3-term bf16 split of |src_m|^2 (x) ones
  PE accumulates in fp32, so the full squared distance (small, >=0) is formed
  before any rounding.  Each [128, 2048] PSUM chunk then flows through two
  parallel engine streams in a per-chunk pipeline:
    - Act: PSUM -> SBUF bf16 evacuation (the only engine that can stream
      PSUM out at 1 elem/cycle/lane; ~122 us/core).
    - DVE: row-min via two fused tensor_scalar min+accum ops per chunk
      (4x mode; 1024-wide accum_out - 2048-wide accums stall real HW), plus one
      full-width running TT-min per block into the [128, 4096] col
      accumulator (split per-chunk for the last block so the tail starts
      early).  Block 0 is evacuated directly into the accumulator.
  Inputs are DMA'd in split pieces (first block/chunk operands first) so the
  PE starts ~2.5us earlier.  Tail: per column-half 32x32 stream transpose +
  TT-min halving of the 32-groups into a [128, 128] tile that is shipped to
  the host, which does the final cross-quadrant fold + sqrt/mean (avoids a
  serial DMA-transpose + fold chain at the very end of the kernel).
"""

import numpy as np
import ml_dtypes
from contextlib import ExitStack

B, C = 8, 3
M = N = 4096
NCORES = 8
PB = 128          # output partition block (m rows per matmul)
KAUG = 18         # augmented contraction dim
BIG = 1.0e4       # > max possible squared distance (~150)
MMN = 512         # matmul moving free dim (one fp32 PSUM bank)
PW = 2048         # psum chunk width (fp32, 4 banks)
SB_BUFS = 8
NBLK = M // PB

# Tunables.  SPLIT_EVAC: blocks whose chunk-1 evacuation runs as a fused DVE
# tensor_scalar straight from PSUM (1x; evac + row-min in one op) while Act
# handles chunk 0 — relieves the Act bottleneck at some DVE cost.
SPLIT_EVAC = ()
ROW_SPLIT = 2               # row-min accums per 2048-chunk ("ts" mode);
                            # 1024-wide accum_out measures penalty-free on
                            # real HW (the accum cliff is at 2048), and
                            # halving the op count saves ~21us/rep of
                            # per-op overhead vs ROW_SPLIT=4
ROW_MODE = "ts"             # "ts": tensor_scalar+accum per chunk; "fold":
FOLD_TO = 512               #   TT-min halving chain down to FOLD_TO + accum
REDUCE_MODE = "ttfold"      # tail 32-group reduce: "reduce" | "ttfold"
SLOTS = 2 * (ROW_SPLIT if ROW_MODE == "ts" else 1)  # rowmin slots per block
VERSION = "balance-v3"

bf16np = ml_dtypes.bfloat16


# ----------------------------------------------------------------------------
# Device program
# ----------------------------------------------------------------------------

def _body(ctx, tc, lhs, rhs, rowmin_d, colmin_d, m, n, reps=1):
    import concourse.mybir as mybir

    nc = tc.nc
    f32 = mybir.dt.float32
    bf16 = mybir.dt.bfloat16
    MIN = mybir.AluOpType.min
    AX = mybir.AxisListType.X

    nblk = m // PB
    pw = min(PW, n)
    nch = n // pw
    mmn = min(MMN, pw)
    nq = pw // mmn

    split_evac = set(b for b in SPLIT_EVAC if b < nblk)
    assert 0 not in split_evac  # block 0 initializes bacc via Act

    cpool = ctx.enter_context(tc.tile_pool(name="const", bufs=1))
    ppool = ctx.enter_context(tc.tile_pool(name="psum", bufs=2, space="PSUM"))
    spool = ctx.enter_context(tc.tile_pool(name="sb", bufs=SB_BUFS))
    xpool = ctx.enter_context(tc.tile_pool(name="scr", bufs=2))
    rpool = ctx.enter_context(tc.tile_pool(name="red", bufs=2))

    # Split the input DMAs so block-0/chunk-0 operands land first and the
    # PE can start ~2.5us earlier than with monolithic transfers.
    lhs_t = cpool.tile([KAUG, m], bf16)
    nc.gpsimd.dma_start(out=lhs_t[:, 0:PB], in_=lhs[:, 0:PB])
    rhs_t = cpool.tile([KAUG, n], bf16)
    nc.sync.dma_start(out=rhs_t[:, 0:mmn], in_=rhs[:, 0:mmn])
    nc.sync.dma_start(out=rhs_t[:, mmn:pw], in_=rhs[:, mmn:pw])
    nc.sync.dma_start(out=lhs_t[:, PB:m], in_=lhs[:, PB:m])
    nc.sync.dma_start(out=rhs_t[:, pw:n], in_=rhs[:, pw:n])

    rowmin_t = cpool.tile([PB, SLOTS * nblk], f32)
    nc.vector.memset(rowmin_t[:], BIG)
    bacc = cpool.tile([PB, n], bf16)

    def row_ts(dchunk, i, ch):
        """Row-min of an evacuated bf16 chunk: fused TS min-reduce at 4x
        (optionally after a TT fold chain, or split into sub-accums)."""
        base = SLOTS * i + ch * (SLOTS // nch)
        if ROW_MODE == "fold":
            src_t, w = dchunk, pw
            lvl = 0
            while w > FOLD_TO:
                nxt = xpool.tile([PB, w // 2], bf16, tag=f"fold{lvl}")
                nc.vector.tensor_tensor(nxt[:], src_t[:, :w // 2],
                                        src_t[:, w // 2:w], MIN)
                src_t, w, lvl = nxt, w // 2, lvl + 1
            scr = xpool.tile([PB, w], bf16, tag="scrf")
            nc.vector.tensor_scalar(
                scr[:], src_t[:, 0:w], float(BIG), None, MIN, MIN,
                accum_out=rowmin_t[:, base:base + 1])
        else:
            scr = xpool.tile([PB, pw], bf16, tag="scr")
            rw = pw // ROW_SPLIT
            for s in range(ROW_SPLIT):
                nc.vector.tensor_scalar(
                    scr[:, s * rw:(s + 1) * rw],
                    dchunk[:, s * rw:(s + 1) * rw], float(BIG), None, MIN,
                    MIN, accum_out=rowmin_t[:, base + s:base + s + 1])

    red_both = cpool.tile([PB, (n // pw) * (pw // 32)], bf16)

    def tail_half(ch):
        """32-group partition min of bacc[:, chunk ch] -> red_both slice."""
        w = pw
        nb = w // 32
        csl = slice(ch * pw, (ch + 1) * pw)
        tr = rpool.tile([PB, w], bf16, tag=f"tr{ch}")
        nc.vector.transpose(tr[:], bacc[:, csl])
        red = red_both[:, ch * nb:(ch + 1) * nb]
        if REDUCE_MODE == "ttfold":
            srcv = tr[:].rearrange("p (b i) -> p b i", i=32)
            wi, lvl = 32, 0
            while wi > 1:
                half = wi // 2
                if half > 1:
                    dst = rpool.tile([PB, nb * half], bf16,
                                     tag=f"tf{ch}_{lvl}",
                                     name=f"tf{ch}_{lvl}")[:]
                else:
                    dst = red
                dstv = dst.rearrange("p (b i) -> p b i", i=half)
                nc.vector.tensor_tensor(dstv, srcv[:, :, 0:half],
                                        srcv[:, :, half:wi], MIN)
                srcv, wi, lvl = dstv, half, lvl + 1
        else:
            nc.vector.tensor_reduce(
                red, tr[:].rearrange("p (b i) -> p b i", i=32), AX, MIN)


    for rep in range(reps):
        for i in range(nblk):
            last = i == nblk - 1 and rep == reps - 1
            for ch in range(nch):
                pt = ppool.tile([PB, pw], f32, tag="pt")
                for q in range(nq):
                    n0 = ch * pw + q * mmn
                    nc.tensor.matmul(
                        pt[:, q * mmn:(q + 1) * mmn],
                        lhs_t[:, i * PB:(i + 1) * PB],
                        rhs_t[:, n0:n0 + mmn],
                        start=True, stop=True,
                    )
                if i == 0:
                    dest = bacc
                elif ch == 0:
                    dest = spool.tile([PB, n], bf16, tag="sb")
                dchunk = dest[:, ch * pw:(ch + 1) * pw]
                if i in split_evac and ch == 1:
                    # fused evac + row-min in one 1x DVE op (relieves Act)
                    base = SLOTS * i + SLOTS // nch
                    nc.vector.tensor_scalar(
                        dchunk[:], pt[:], float(BIG), None, MIN, MIN,
                        accum_out=rowmin_t[:, base:base + 1])
                else:
                    nc.scalar.copy(dchunk[:], pt[:])
                    row_ts(dchunk, i, ch)
                if i > 0 and (last or ch == nch - 1):
                    if last:
                        # per-chunk col TT so tail_half(0) can start early
                        csl = slice(ch * pw, (ch + 1) * pw)
                        nc.vector.tensor_tensor(bacc[:, csl], bacc[:, csl],
                                                dchunk[:], MIN)
                    else:
                        # one merged full-width col TT (fewer DVE ops)
                        nc.vector.tensor_tensor(bacc[:], bacc[:], dest[:],
                                                MIN)
                if last and ch == 0:
                    tail_half(0)
            if last:
                nc.sync.dma_start(out=rowmin_d[:], in_=rowmin_t[:])
                tail_half(1)
                # the cross-quadrant fold of red_both happens on the host;
                # shipping the [128, 128] pre-fold tile avoids a serial
                # DMA-transpose + fold chain at the very end of the kernel.
                nc.sync.dma_start(out=colmin_d[:], in_=red_both[:])


def build_nc(m=M, n=N, reps=1):
    import concourse.tile as tile
    import concourse.bacc as bacc_mod
    import concourse.mybir as mybir

    f32 = mybir.dt.float32
    bf16 = mybir.dt.bfloat16
    nblk = m // PB

    nc = bacc_mod.Bacc("TRN2", target_bir_lowering=False, debug=False)
    lhs = nc.dram_tensor("lhs_aug", [KAUG, m], bf16, kind="ExternalInput").ap()
    rhs = nc.dram_tensor("rhs_aug", [KAUG, n], bf16, kind="ExternalInput").ap()
    rowmin_d = nc.dram_tensor("rowmin", [PB, SLOTS * nblk], f32,
                              kind="ExternalOutput").ap()
    colmin_d = nc.dram_tensor("colmin", [PB, PB], bf16,
                              kind="ExternalOutput").ap()
    with tile.TileContext(nc) as tc:
        with ExitStack() as ctx:
            _body(ctx, tc, lhs, rhs, rowmin_d, colmin_d, m, n, reps=reps)
    nc.compile()
    return nc


# ----------------------------------------------------------------------------
# Host-side input prep: exact bf16 splits for the augmented operands
# ----------------------------------------------------------------------------

def _split2(x):
    hi = x.astype(bf16np).astype(np.float64)
    lo = (x - hi).astype(bf16np).astype(np.float64)
    return hi, lo


def _split3(x):
    h = x.astype(bf16np).astype(np.float64)
    r = x - h
    mdl = r.astype(bf16np).astype(np.float64)
    l = (r - mdl).astype(bf16np).astype(np.float64)
    return h, mdl, l


def prep_inputs(pc_src, pc_dst):
    """Build per-batch augmented operands L, R: [B, 18, M/N] bf16."""
    s = np.asarray(pc_src, dtype=np.float64)   # [B, 3, M]
    d = np.asarray(pc_dst, dtype=np.float64)   # [B, 3, N]
    b = s.shape[0]
    m = s.shape[2]
    n = d.shape[2]

    s_hi, s_lo = _split2(s)
    d_hi, d_lo = _split2(d)
    s2 = ((s_hi + s_lo) ** 2).sum(axis=1)      # [B, M]
    d2 = ((d_hi + d_lo) ** 2).sum(axis=1)      # [B, N]
    s2h, s2m, s2l = _split3(s2)
    d2h, d2m, d2l = _split3(d2)

    L = np.zeros((b, KAUG, m), dtype=np.float64)
    R = np.zeros((b, KAUG, n), dtype=np.float64)
    L[:, 0:3] = -2.0 * s_hi
    R[:, 0:3] = d_hi
    L[:, 3:6] = -2.0 * s_hi
    R[:, 3:6] = d_lo
    L[:, 6:9] = -2.0 * s_lo
    R[:, 6:9] = d_hi
    L[:, 9:12] = -2.0 * s_lo
    R[:, 9:12] = d_lo
    L[:, 12:15] = 1.0
    R[:, 12] = d2h
    R[:, 13] = d2m
    R[:, 14] = d2l
    L[:, 15] = s2h
    L[:, 16] = s2m
    L[:, 17] = s2l
    R[:, 15:18] = 1.0
    return L.astype(bf16np), R.astype(bf16np)


# ----------------------------------------------------------------------------
# Cached PJRT runner (compile once, execute many)
# ----------------------------------------------------------------------------

_STATE = {}


def _get_runner(reps=1):
    key = (reps, VERSION, SB_BUFS, SPLIT_EVAC, ROW_MODE, ROW_SPLIT,
           FOLD_TO, REDUCE_MODE)
    if key in _STATE:
        return _STATE[key]

    import jax
    from jax.experimental.shard_map import shard_map
    from jax.sharding import Mesh, PartitionSpec
    from concourse import bass2jax, mybir

    nc = build_nc(M, N, reps=reps)
    bass2jax.install_neuronx_cc_hook()

    in_names, in_shapes, out_names, out_avals = [], {}, [], []
    for alloc in nc.m.functions[0].allocations:
        if not isinstance(alloc, mybir.MemoryLocationSet):
            continue
        name = alloc.memorylocations[0].name
        if alloc.kind == "ExternalInput":
            in_names.append(name)
            in_shapes[name] = (tuple(alloc.tensor_shape),
                               mybir.dt.np(alloc.dtype))
        elif alloc.kind == "ExternalOutput":
            out_names.append(name)
            out_avals.append(jax.core.ShapedArray(
                tuple(alloc.tensor_shape), mybir.dt.np(alloc.dtype)))
    n_params = len(in_names)
    n_outs = len(out_names)
    all_in_names = tuple(in_names + out_names)
    donate = tuple(range(n_params, n_params + n_outs))

    def _jbody(*args):
        outs = bass2jax._bass_exec_p.bind(
            *args,
            out_avals=tuple(out_avals),
            in_names=all_in_names,
            out_names=tuple(out_names),
            lowering_input_output_aliases=(),
            sim_require_finite=True,
            sim_require_nnan=True,
            nc=nc,
        )
        return tuple(outs)

    devices = jax.devices()[:NCORES]
    mesh = Mesh(np.asarray(devices), ("core",))
    in_specs = (PartitionSpec("core"),) * (n_params + n_outs)
    out_specs = (PartitionSpec("core"),) * n_outs
    fn = jax.jit(
        shard_map(_jbody, mesh=mesh, in_specs=in_specs, out_specs=out_specs,
                  check_rep=False),
        donate_argnums=donate, keep_unused=True,
    )
    st = dict(fn=fn, nc=nc, in_names=in_names, in_shapes=in_shapes,
              out_names=out_names, out_avals=out_avals, n_params=n_params)
    _STATE[key] = st
    return st


def run_device(L, R, reps=1, _retry=True):
    """L, R: [NCORES, 18, M] bf16. Returns (rowmin[NCORES,128,2*M/128],
    colmin[NCORES,32,N/32]) squared-distance minima (fp32)."""
    st = _get_runner(reps)
    concat_in = []
    for name in st["in_names"]:
        if name == "lhs_aug":
            arr = L
        elif name == "rhs_aug":
            arr = R
        else:  # e.g. partition_id — framework-added aux input
            shape, dt = st["in_shapes"][name]
            concat_in.append(np.zeros((NCORES * shape[0], *shape[1:]), dt))
            continue
        concat_in.append(np.concatenate([arr[c] for c in range(NCORES)], axis=0))
    concat_zeros = [
        np.zeros((NCORES * av.shape[0], *av.shape[1:]), av.dtype)
        for av in st["out_avals"]
    ]
    try:
        out_arrs = st["fn"](*concat_in, *concat_zeros)
        out_np = [np.asarray(a) for a in out_arrs]
    except Exception:
        # The shared axon terminal occasionally reports a transient
        # device-unrecoverable state; it clears after a short pause.
        if not _retry:
            raise
        import time as _time
        _time.sleep(20.0)
        return run_device(L, R, reps=reps, _retry=False)
    outs = {}
    for i, name in enumerate(st["out_names"]):
        av = st["out_avals"][i]
        outs[name] = out_np[i].reshape(NCORES, *av.shape)
    return outs["rowmin"], outs["colmin"]


# ----------------------------------------------------------------------------
# Public entry point
# ----------------------------------------------------------------------------

def _host_reduce(rowmin, colmin):
    # rowmin: [B, 128, 2*nblk] (per-half-chunk row minima); colmin: [B, 32,
    # N/32] (squared distances)
    b = rowmin.shape[0]
    rm = rowmin.reshape(b, PB, NBLK, -1).min(axis=3)
    fwd = np.sqrt(np.maximum(rm.astype(np.float64), 0.0)).mean()
    # colmin arrives pre-fold as [128, 128]: partition = (quadrant, s),
    # free = 32-column-group; fold the 4 partition quadrants here.
    cq = colmin.astype(np.float64).reshape(b, 4, 32, PB).min(axis=1)
    bwd = np.sqrt(np.maximum(cq, 0.0)).mean()
    total = np.float32(fwd + bwd)
    return total


def kernel(pc_src, pc_dst):
    L, R = prep_inputs(pc_src, pc_dst)
    rowmin, colmin = run_device(L, R)
    total = _host_reduce(rowmin, colmin)
    return (total, total, total)
